# revision 38
# baseline (speedup 1.0000x reference)
"""DTW loss (symmetric2, unnormalized) on trn2 — bidirectional 2-core kernel
with a partition-skewed diagonal-wavefront DP.

kernel(x, y) -> np.float32 scalar DTW distance, matching the jax reference:
  D[0,0]=c[0,0]; D[0,j]=D[0,j-1]+c; D[i,0]=D[i-1,0]+c
  D[i,j]=min(D[i-1,j-1]+2c, D[i-1,j]+c, D[i,j-1]+c);  c = euclidean cdist.

Core 0 runs the forward DP over rows [0,2048); core 1 runs the backward
table H (reverse DP, entering-convention) on reversed inputs. Both run the
SAME program (see _host_prep for the shifted-cost-matrix trick). The host
merges the two meeting rows.

Phase A computes the cost matrices by PE matmul (K=65 augmented with norm
rows) + ScalarE sqrt, staged to DRAM as [slot][12288] with slot = row+127
(padding slots hold BIG): cols [0,8192) interleave (2*C2, C1) per column
(2*C2 = sqrt(4 d^2) via scale=-8/bias 4|xs|^2), cols [8192,12288) = C3.

Phase B is a diagonal wavefront: at step s, partition p handles segment
q = 127-p (columns [32q, 32q+32)) of DP row r = s-127+p. The 32-wide
min-plus scan per partition is exact given a per-partition init (the left
boundary D[r, 32q-1]), which equals the segment-end of partition p+1 at
step s-1. That cross-partition move runs on the otherwise-idle PE as a
subdiagonal-matrix matmul into PSUM; the next scan reads its init straight
from PSUM. The scan covers 33 elements: element 0 re-emits the init into
the D buffer's boundary column (data0=0, data1=BIG there), so the next
row's paired read (D[r-1,j-1], D[r-1,j]) is one overlapping-stride AP.
Per step the DVE runs only 3 chained ops — paired add, pairwise
tensor_reduce(min), scan — at 733 ns/step vs the 9-op/1857 ns row loop of
the row-sequential version. That is this structure's latency floor: the
chain (3 sem hops + 3 ops) costs 633 ns and the scan's second wait class
(PSUM init + same-engine RAW, ISA allows one inline wait) forces a
split EventSemaphore worth +100 ns; the PE round trip itself is fully
hidden. Phase A drips into the step stream as 128-col matmul+sqrt chunks
(two per step) and loads prefetch 3 batches ahead, so the in-order
PE/SP/DMA engines never head-of-line-block the wavefront.
"""

import sys

sys.path.insert(0, "/opt/trn_rl_repo")

import numpy as np

N = 4096          # rows of x
M = 4096          # rows of y
DIM = 64
N_ROWS = N // 2   # DP rows per core
P = 128           # partitions in the wavefront
SEG = 32          # columns per segment (M // P)
STEPS = N_ROWS + P - 1
SLOT = 3 * M      # floats per DP-row slot in SI
N_SLOTS = P + N_ROWS + 129  # 127 head pads, rows, tail pads (incl. load overrun)
BIG = 1e30
BIGW = 1e36       # shift-matrix weight that maps any end value to ~inf

_nc_cache = {}


def _build_nc(n_rows=N_ROWS, dp_rows=None):
    """dp_rows: run phase B over only this many rows (timing experiments)."""
    if dp_rows is None:
        dp_rows = n_rows
    key = (n_rows, dp_rows)
    if key in _nc_cache:
        return _nc_cache[key]
    import concourse.bacc as bacc
    import concourse.bass as bass
    import concourse.mybir as mybir
    from concourse import tile

    F32 = mybir.dt.float32
    AluOp = mybir.AluOpType
    n_bands = n_rows // 128
    steps = dp_rows + P - 1
    n_batches = (steps + 7) // 8

    nc = bacc.Bacc(None, target_bir_lowering=False)

    xt_d = nc.dram_tensor("xt", [65, n_rows], F32, kind="ExternalInput")
    xst_d = nc.dram_tensor("xst", [65, n_rows], F32, kind="ExternalInput")
    yt_d = nc.dram_tensor("yt", [65, M], F32, kind="ExternalInput")
    yst_d = nc.dram_tensor("yst", [65, M], F32, kind="ExternalInput")
    xx_d = nc.dram_tensor("xx", [n_rows], F32, kind="ExternalInput")
    xxs_d = nc.dram_tensor("xxs", [n_rows], F32, kind="ExternalInput")
    xxs4_d = nc.dram_tensor("xxs4", [n_rows], F32, kind="ExternalInput")
    # [:, 0:128] subdiagonal shift (+BIGW hook), [:, 128:256] identity
    shiftm_d = nc.dram_tensor("shiftm", [128, 256], F32, kind="ExternalInput")
    initc_d = nc.dram_tensor("initc", [128], F32, kind="ExternalInput")
    out_d = nc.dram_tensor("out", [M], F32, kind="ExternalOutput")
    SI = nc.dram_tensor("SI", [N_SLOTS * SLOT], F32)

    with tile.TileContext(nc) as tc:
        with (
            tc.tile_pool(name="const", bufs=1) as constp,
            tc.tile_pool(name="band", bufs=2) as bandp,
            tc.tile_pool(name="psumA", bufs=4, space="PSUM") as psumA,
            tc.tile_pool(name="psumS", bufs=2, space="PSUM") as psumS,
        ):
            xt_sb = constp.tile([65, n_rows], F32, tag="xt")
            xst_sb = constp.tile([65, n_rows], F32, tag="xst")
            yt_sb = constp.tile([65, M], F32, tag="yt")
            yst_sb = constp.tile([65, M], F32, tag="yst")
            xx_sb = constp.tile([128, n_bands], F32, tag="xx")
            xxs_sb = constp.tile([128, n_bands], F32, tag="xxs")
            xxs4_sb = constp.tile([128, n_bands], F32, tag="xxs4")
            shift_sb = constp.tile([128, 256], F32, tag="shiftm")
            bigpad = constp.tile([128, 1536], F32, tag="bigpad")
            initcol = constp.tile([P, 1], F32, tag="initcol")
            uint_t = constp.tile([P, 2 * SEG], F32, tag="uint_t")
            btile = constp.tile([P, SEG + 1], F32, tag="btile")
            dbuf0 = constp.tile([P, SEG + 1], F32, tag="dbuf0")
            dbuf1 = constp.tile([P, SEG + 1], F32, tag="dbuf1")
            scr0 = constp.tile([P, 1], F32, tag="scr0")
            scr1 = constp.tile([P, 1], F32, tag="scr1")
            outrow = constp.tile([1, M], F32, tag="outrow")
            # 4 batch buffers: 3-deep prefetch of 8-step diagonal loads
            ambufs = [
                constp.tile([P, 8 * 2 * SEG], F32, tag=f"am{i}", name=f"am{i}")
                for i in range(4)
            ]
            gbufs = [
                constp.tile([P, 8 * 33 + 1], F32, tag=f"g{i}", name=f"g{i}")
                for i in range(4)
            ]

            nc.sync.dma_start(xst_sb[:], xst_d[:])
            nc.sync.dma_start(yt_sb[:], yt_d[:])
            nc.sync.dma_start(
                xxs_sb[:], xxs_d[:].rearrange("(b p) -> p b", p=128)
            )
            nc.sync.dma_start(yst_sb[:], yst_d[:])
            nc.sync.dma_start(xt_sb[:], xt_d[:])
            nc.sync.dma_start(xx_sb[:], xx_d[:].rearrange("(b p) -> p b", p=128))
            nc.sync.dma_start(
                xxs4_sb[:], xxs4_d[:].rearrange("(b p) -> p b", p=128)
            )
            nc.sync.dma_start(shift_sb[:], shiftm_d[:])
            nc.sync.dma_start(
                initcol[:], initc_d[:].rearrange("(p f) -> p f", f=1)
            )
            nc.gpsimd.memset(bigpad[:], BIG)
            nc.gpsimd.memset(dbuf0[:], BIG)
            nc.gpsimd.memset(dbuf1[:], BIG)
            nc.gpsimd.memset(btile[:, 0:1], BIG)
            for gb in gbufs:
                nc.gpsimd.memset(
                    gb[:, 0:264].rearrange("p (k f) -> p k f", f=33)[:, :, 0:1],
                    0.0,
                )

            # BIG padding for head slots [0,127) and tail slots [127+dp_rows,
            # N_SLOTS): garbage wavefront cells read/write values in
            # [BIG, ~130*BIG] — finite, and always losing min() against real
            # path values.
            def fill_pad(slot0, slot1):
                total = (slot1 - slot0) * SLOT  # multiple of 1536 (SLOT is)
                off = slot0 * SLOT
                chunk = 128 * 1536
                while total > 0:
                    n = min(chunk, total)
                    rows = n // 1536
                    nc.sync.dma_start(
                        SI[off:off + n].rearrange("(p f) -> p f", f=1536),
                        bigpad[0:rows, :],
                    )
                    off += n
                    total -= n

            assert SLOT % 1536 == 0
            fill_pad(0, 127)

            # ---- phase A band: three cost matrices for rows [128b, 128b+128)
            # Band layout [128, SLOT]: cols [0,8192) interleave (2*C2, C1)
            # per column — (even, odd) = the diagonal/vertical operand pair
            # phase B consumes with one paired add — cols [8192,12288) = C3.
            # 2*C2 = sqrt(4*d^2) comes free via scale=-8 & bias 4*|xs|^2.
            # Emitted as per-step closures (one 128-col matmul+sqrt chunk)
            # so the in-order PE never buries the wavefront's shift matmuls
            # under a burst of band matmuls.
            passes = (
                (0, xst_sb, xxs_sb, yt_sb, -2.0),
                (1, xst_sb, xxs4_sb, yst_sb, -8.0),
                (2, xt_sb, xx_sb, yst_sb, -2.0),
            )

            def band_chunks(b):
                c_band = bandp.tile([128, SLOT], F32, tag="c_band", name="cb")
                am_view = c_band[:, 0:2 * M].rearrange("p (c e) -> p c e", e=2)

                def chunk(t, lhs, bias_sb, rhs, scale, cb):
                    ps = psumA.tile([128, 128], F32, tag="psA", name="psA")
                    nc.tensor.matmul(
                        ps[:],
                        lhs[:, b * 128:(b + 1) * 128],
                        rhs[:, cb * 128:(cb + 1) * 128],
                    )
                    if t == 2:
                        out = c_band[:, 2 * M + cb * 128:2 * M + (cb + 1) * 128]
                    else:
                        out = am_view[:, cb * 128:(cb + 1) * 128, (1 - t):(2 - t)]
                    nc.scalar.activation(
                        out, ps[:], mybir.ActivationFunctionType.Sqrt,
                        bias=bias_sb[:, b:b + 1], scale=scale,
                    )

                def write(qtr):
                    si_dst = bass.AP(
                        SI[:].tensor,
                        (b * 128 + qtr * 32 + 127) * SLOT,
                        [[SLOT, 32], [1, SLOT]],
                    )
                    nc.sync.dma_start(
                        si_dst, c_band[qtr * 32:(qtr + 1) * 32, :]
                    )

                for t, lhs, bias_sb, rhs, scale in passes:
                    for cb in range(32):
                        yield (lambda t=t, lhs=lhs, bias_sb=bias_sb, rhs=rhs,
                               scale=scale, cb=cb: chunk(t, lhs, bias_sb, rhs,
                                                         scale, cb))
                for qtr in range(4):
                    yield (lambda qtr=qtr: write(qtr))

            def emit_band(b):
                for fn in band_chunks(b):
                    fn()

            # ---- phase B batch load: 8 diagonal steps per DMA trio ----
            # partition p <-> segment q=127-p; step s, partition p reads
            # (row s-127+p, seg q) = slot s+p at column offset 32q.
            def emit_load(batch):
                S = batch * 8
                amt = ambufs[batch % 4]
                gt_ = gbufs[batch % 4]
                nc.sync.dma_start(
                    amt[:].rearrange("p (k f) -> p k f", f=2 * SEG),
                    bass.AP(
                        SI[:].tensor, S * SLOT + (P - 1) * 2 * SEG,
                        [[SLOT - 2 * SEG, 128], [SLOT, 8], [1, 2 * SEG]],
                    ),
                )
                nc.sync.dma_start(
                    gt_[:, 1:265].rearrange("p (k f) -> p k f", f=33)[:, :, 0:32],
                    bass.AP(
                        SI[:].tensor, S * SLOT + 2 * M + (P - 1) * SEG,
                        [[SLOT - SEG, 128], [SLOT, 8], [1, SEG]],
                    ),
                )

            # Band 0 is on the startup critical path: its chunks run upfront, and only the quarter-0 writes (rows
            # 0-31 — all the first ~3 loads read) go ahead of the loads.
            c_band0 = bandp.tile([128, SLOT], F32, tag="c_band", name="cb0")
            am_view0 = c_band0[:, 0:2 * M].rearrange("p (c e) -> p c e", e=2)
            for t, lhs, bias_sb, rhs, scale in passes:
                for cb in range(32):
                    ps = psumA.tile([128, 128], F32, tag="psA", name="psA")
                    nc.tensor.matmul(
                        ps[:], lhs[:, 0:128], rhs[:, cb * 128:(cb + 1) * 128]
                    )
                    if t == 2:
                        out = c_band0[:, 2 * M + cb * 128:2 * M + (cb + 1) * 128]
                    else:
                        out = am_view0[:, cb * 128:(cb + 1) * 128, (1 - t):(2 - t)]
                    nc.scalar.activation(
                        out, ps[:], mybir.ActivationFunctionType.Sqrt,
                        bias=bias_sb[:, 0:1], scale=scale,
                    )
                if t == 1:
                    nc.sync.dma_start(
                        bass.AP(SI[:].tensor, 127 * SLOT, [[SLOT, 32], [1, 2 * M]]),
                        c_band0[0:32, 0:2 * M],
                    )
            nc.sync.dma_start(
                bass.AP(SI[:].tensor, 127 * SLOT + 2 * M, [[SLOT, 32], [1, M]]),
                c_band0[0:32, 2 * M:SLOT],
            )
            # Row 0's boundary column carries the virtual D[0,-1]=0 cumsum
            # seed, which row 1's diagonal term must NOT see as a real cell:
            # overwrite 2*C2[1,0] with BIG so that single path is cut exactly.
            nc.sync.dma_start(
                SI[128 * SLOT:128 * SLOT + 1].rearrange("(p f) -> p f", f=1),
                bigpad[0:1, 0:1],
            )
            emit_load(0)
            emit_load(1)
            emit_load(2)

            def band0_write(qtr):
                nc.sync.dma_start(
                    bass.AP(
                        SI[:].tensor, (qtr * 32 + 127) * SLOT,
                        [[SLOT, 32], [1, SLOT]],
                    ),
                    c_band0[qtr * 32:(qtr + 1) * 32, :],
                )

            for qtr in range(1, 4):
                band0_write(qtr)

            # ---- phase B: diagonal wavefront ----
            # Remaining bands drip into the step stream, two 128-col
            # matmul+sqrt chunks per step (~2.5 rows produced per row
            # consumed) so loads never wait on band writes; band 1 gets a
            # head start during the startup window (PE is idle then)
            pending = []
            for b in range(1, n_bands):
                pending.extend(band_chunks(b))
            pending.reverse()  # pop() from the tail emits in order

            dbufs = [dbuf0, dbuf1]
            scrs = [scr0, scr1]
            dtens = [dbuf0[:].tensor, dbuf1[:].tensor]
            ps_prev = None
            for s in range(steps):
                if s >= 32:
                    # drip starts after the wavefront is rolling: the
                    # scheduler hoists ready chunk matmuls ahead of blocked
                    # shifts, and early scans (counting sems) would wait for
                    # that whole Act-paced backlog. 2/step until bands lead
                    # comfortably, then 1/step to cut PE counting-sem
                    # coupling jitter in the overlap region.
                    for _ in range(2 if s < 256 else 1):
                        if pending:
                            pending.pop()()
                if s == 1024:
                    fill_pad(127 + dp_rows, N_SLOTS)
                amt = ambufs[(s // 8) % 4]
                gt_ = gbufs[(s // 8) % 4]
                sl = s % 8
                am_s = amt[:, sl * 2 * SEG:(sl + 1) * 2 * SEG]
                g33 = gt_[:, sl * 33:sl * 33 + 33]
                dcur = dbufs[s % 2]
                dprev = dbufs[(s + 1) % 2]
                # paired operands (D[r-1,j-1], D[r-1,j]) via overlapping AP
                dpairs = bass.AP(
                    dtens[(s + 1) % 2], 0, [[SEG + 1, 128], [1, SEG], [1, 2]]
                )
                nc.vector.tensor_tensor(
                    uint_t[:].rearrange("p (c e) -> p c e", e=2),
                    dpairs, am_s.rearrange("p (c e) -> p c e", e=2),
                    op=AluOp.add,
                )
                if s > 0:
                    # stage the init (shift_{s-1}'s PSUM result, ready since
                    # last step) into SBUF here: this copy's waits are all
                    # satisfied, so it absorbs the 2-wait-class
                    # EventSemaphore cheaply and the scan keeps a single
                    # DVE-class wait — no +100ns ES on the critical tail
                    nc.vector.tensor_copy(scrs[s % 2][:, 0:1], ps_prev[:, 0:1])
                nc.vector.tensor_reduce(
                    btile[:, 1:SEG + 1],
                    uint_t[:].rearrange("p (c e) -> p c e", e=2),
                    axis=mybir.AxisListType.X, op=AluOp.min,
                )
                init = initcol[:, 0:1] if s == 0 else scrs[s % 2][:, 0:1]
                nc.vector.tensor_tensor_scan(
                    dcur[:, 0:SEG + 1], g33, btile[:, 0:SEG + 1], init,
                    op0=AluOp.add, op1=AluOp.min,
                )
                if s + 1 < steps:
                    ps_cur = psumS.tile([128, 1], F32, tag="shift")
                    nc.tensor.matmul(
                        ps_cur[:], shift_sb[:, 0:128], dcur[:, SEG:SEG + 1]
                    )
                    ps_prev = ps_cur
                if s % 8 == 0 and s // 8 + 3 < n_batches:
                    # end-of-step emission: batch-boundary scans wait (via
                    # the per-queue counting sems) on every earlier same-
                    # queue DMA, so never put a fresh load ahead of a scan
                    emit_load(s // 8 + 3)
                if s >= dp_rows - 1:
                    # extract partition p's last-row segment via a one-hot
                    # PE matmul (engine ops can't start at partition p)
                    p = dp_rows - 1 + P - 1 - s
                    q = P - 1 - p
                    pse = psumS.tile([128, SEG], F32, tag="ext")
                    nc.tensor.matmul(
                        pse[0:1, 0:SEG],
                        shift_sb[:, 128 + p:129 + p],
                        dcur[:, 1:SEG + 1],
                    )
                    nc.vector.tensor_copy(
                        outrow[0:1, q * SEG:(q + 1) * SEG], pse[0:1, 0:SEG]
                    )

            nc.sync.dma_start(
                out_d[:].rearrange("(p f) -> p f", p=1), outrow[:]
            )

    nc.compile()
    _nc_cache[key] = nc
    return nc


def _aug_t(a):
    """[n, 64] -> [65, n] transposed + ones row (f32)."""
    n = a.shape[0]
    t = np.ones((65, n), np.float32)
    t[:64] = a.T
    return t


def _aug_y(yf):
    """[m, 64] -> [65, m]: y^T with row 64 = -0.5*(|y|^2 + 1e-12)."""
    t = np.empty((65, yf.shape[0]), np.float32)
    t[:64] = yf.T
    yy = (yf.astype(np.float64) ** 2).sum(1)
    t[64] = (-0.5 * (yy + 1e-12)).astype(np.float32)
    return t


def _pe_consts():
    """[:, 0:128]: lhsT so out[m] = ends[m+1]; partition 127 (segment 0)
    instead gets BIGW * ends[0] ~ +inf (its left boundary is outside the
    matrix). [:, 128:256]: identity for one-hot row extraction."""
    sm = np.zeros((128, 256), np.float32)
    for m_ in range(127):
        sm[m_ + 1, m_] = 1.0
    sm[0, 127] = BIGW
    sm[:, 128:256] = np.eye(128, dtype=np.float32)
    return sm


def _init_col():
    col = np.full(128, BIG, np.float32)
    col[127] = 0.0  # virtual D[0,-1] = 0 seeds row 0's cumsum
    return col


def _host_prep(xh, yf, core):
    if core == 0:
        xs, ys = xh, yf
    else:
        xs = np.concatenate([np.zeros((1, DIM), np.float32), xh[:-1]])
        # ys[0] pairs with x[0] to make C3[0,0] ~ 0 (H[0,0]=0). Perturb it
        # so the fp32-cancelled squared distance stays safely positive
        # (sqrt of a slightly negative value would poison the DP with NaN);
        # the 0.1 offset costs ~1e-6 relative error on the final answer.
        y0 = xh[0:1].copy()
        y0[0, 0] += 0.1
        ys = np.concatenate([y0, yf[:-1]])
    xxs = (xs.astype(np.float64) ** 2).sum(1)
    return {
        "xt": _aug_t(xh),
        "xst": _aug_t(xs),
        "yt": _aug_y(yf),
        "yst": _aug_y(ys),
        "xx": (xh.astype(np.float64) ** 2).sum(1).astype(np.float32),
        "xxs": xxs.astype(np.float32),
        "xxs4": (4.0 * xxs).astype(np.float32),
        "shiftm": _pe_consts(),
        "initc": _init_col(),
    }


def kernel(x, y):
    x = np.ascontiguousarray(np.asarray(x, dtype=np.float32))
    y = np.ascontiguousarray(np.asarray(y, dtype=np.float32))
    assert x.shape == (N, DIM) and y.shape == (M, DIM)

    from concourse.bass_utils import run_bass_kernel_spmd

    nc = _build_nc()
    in_maps = [
        _host_prep(x[:N_ROWS], y, core=0),
        _host_prep(x[::-1][:N_ROWS].copy(), y[::-1].copy(), core=1),
    ]
    res = run_bass_kernel_spmd(nc, in_maps, core_ids=[0, 1])
    F_last = res.results[0]["out"].astype(np.float64)
    H_last = res.results[1]["out"].astype(np.float64)

    # host merge across the row-2047/2048 seam
    xm = x[N_ROWS].astype(np.float64)
    sq = (xm * xm).sum() + (y.astype(np.float64) ** 2).sum(1) - 2.0 * (
        y.astype(np.float64) @ xm
    )
    c_mid = np.sqrt(np.maximum(sq, 1e-12))
    B_row = H_last[::-1]
    cand_v = F_last + c_mid + B_row
    cand_d = F_last[:-1] + 2.0 * c_mid[1:] + B_row[1:]
    ans = min(cand_v.min(), cand_d.min())
    return np.float32(ans)


# revision 39
# speedup vs baseline: 1.0111x; 1.0111x over previous
"""DTW loss (symmetric2, unnormalized) on trn2 — bidirectional 2-core kernel
with a partition-skewed diagonal-wavefront DP.

kernel(x, y) -> np.float32 scalar DTW distance, matching the jax reference:
  D[0,0]=c[0,0]; D[0,j]=D[0,j-1]+c; D[i,0]=D[i-1,0]+c
  D[i,j]=min(D[i-1,j-1]+2c, D[i-1,j]+c, D[i,j-1]+c);  c = euclidean cdist.

Core 0 runs the forward DP over rows [0,2048); core 1 runs the backward
table H (reverse DP, entering-convention) on reversed inputs. Both run the
SAME program (see _host_prep for the shifted-cost-matrix trick). The host
merges the two meeting rows.

Phase A computes the cost matrices by PE matmul (K=65 augmented with norm
rows) + ScalarE sqrt, staged to DRAM as [slot][12288] with slot = row+127
(padding slots hold BIG): cols [0,8192) interleave (2*C2, C1) per column
(2*C2 = sqrt(4 d^2) via scale=-8/bias 4|xs|^2), cols [8192,12288) = C3.

Phase B is a diagonal wavefront: at step s, partition p handles segment
q = 127-p (columns [32q, 32q+32)) of DP row r = s-127+p. The 32-wide
min-plus scan per partition is exact given a per-partition init (the left
boundary D[r, 32q-1]), which equals the segment-end of partition p+1 at
step s-1. That cross-partition move runs on the otherwise-idle PE as a
subdiagonal-matrix matmul into PSUM; the next scan reads its init straight
from PSUM. The scan covers 33 elements: element 0 re-emits the init into
the D buffer's boundary column (data0=0, data1=BIG there), so the next
row's paired read (D[r-1,j-1], D[r-1,j]) is one overlapping-stride AP.
Per step the DVE runs only 3 chained ops — paired add, pairwise
tensor_reduce(min), scan — at 733 ns/step vs the 9-op/1857 ns row loop of
the row-sequential version. That is this structure's latency floor: the
chain (3 sem hops + 3 ops) costs 633 ns and the scan's second wait class
(PSUM init + same-engine RAW, ISA allows one inline wait) forces a
split EventSemaphore worth +100 ns; the PE round trip itself is fully
hidden. Phase A drips into the step stream as 128-col matmul+sqrt chunks
(two per step) and loads prefetch 3 batches ahead, so the in-order
PE/SP/DMA engines never head-of-line-block the wavefront.
"""

import sys

sys.path.insert(0, "/opt/trn_rl_repo")

import numpy as np

N = 4096          # rows of x
M = 4096          # rows of y
DIM = 64
N_ROWS = N // 2   # DP rows per core
P = 128           # partitions in the wavefront
SEG = 32          # columns per segment (M // P)
STEPS = N_ROWS + P - 1
SLOT = 3 * M      # floats per DP-row slot in SI
N_SLOTS = P + N_ROWS + 129  # 127 head pads, rows, tail pads (incl. load overrun)
BIG = 1e30
BIGW = 1e36       # shift-matrix weight that maps any end value to ~inf

_nc_cache = {}


def _build_nc(n_rows=N_ROWS, dp_rows=None):
    """dp_rows: run phase B over only this many rows (timing experiments)."""
    if dp_rows is None:
        dp_rows = n_rows
    key = (n_rows, dp_rows)
    if key in _nc_cache:
        return _nc_cache[key]
    import concourse.bacc as bacc
    import concourse.bass as bass
    import concourse.mybir as mybir
    from concourse import tile

    F32 = mybir.dt.float32
    AluOp = mybir.AluOpType
    n_bands = n_rows // 128
    steps = dp_rows + P - 1
    n_batches = (steps + 7) // 8

    nc = bacc.Bacc(None, target_bir_lowering=False)

    xt_d = nc.dram_tensor("xt", [65, n_rows], F32, kind="ExternalInput")
    xst_d = nc.dram_tensor("xst", [65, n_rows], F32, kind="ExternalInput")
    yt_d = nc.dram_tensor("yt", [65, M], F32, kind="ExternalInput")
    yst_d = nc.dram_tensor("yst", [65, M], F32, kind="ExternalInput")
    xx_d = nc.dram_tensor("xx", [n_rows], F32, kind="ExternalInput")
    xxs_d = nc.dram_tensor("xxs", [n_rows], F32, kind="ExternalInput")
    xxs4_d = nc.dram_tensor("xxs4", [n_rows], F32, kind="ExternalInput")
    # [:, 0:128] subdiagonal shift (+BIGW hook), [:, 128:256] identity
    shiftm_d = nc.dram_tensor("shiftm", [128, 256], F32, kind="ExternalInput")
    initc_d = nc.dram_tensor("initc", [128], F32, kind="ExternalInput")
    out_d = nc.dram_tensor("out", [M], F32, kind="ExternalOutput")
    SI = nc.dram_tensor("SI", [N_SLOTS * SLOT], F32)

    with tile.TileContext(nc) as tc:
        with (
            tc.tile_pool(name="const", bufs=1) as constp,
            tc.tile_pool(name="band", bufs=2) as bandp,
            tc.tile_pool(name="psumA", bufs=4, space="PSUM") as psumA,
            tc.tile_pool(name="psumS", bufs=2, space="PSUM") as psumS,
        ):
            xt_sb = constp.tile([65, n_rows], F32, tag="xt")
            xst_sb = constp.tile([65, n_rows], F32, tag="xst")
            yt_sb = constp.tile([65, M], F32, tag="yt")
            yst_sb = constp.tile([65, M], F32, tag="yst")
            xx_sb = constp.tile([128, n_bands], F32, tag="xx")
            xxs_sb = constp.tile([128, n_bands], F32, tag="xxs")
            xxs4_sb = constp.tile([128, n_bands], F32, tag="xxs4")
            shift_sb = constp.tile([128, 256], F32, tag="shiftm")
            bigpad = constp.tile([128, 1536], F32, tag="bigpad")
            initcol = constp.tile([P, 1], F32, tag="initcol")
            uint_t = constp.tile([P, 2 * SEG], F32, tag="uint_t")
            btile = constp.tile([P, SEG + 1], F32, tag="btile")
            dbuf0 = constp.tile([P, SEG + 1], F32, tag="dbuf0")
            dbuf1 = constp.tile([P, SEG + 1], F32, tag="dbuf1")
            scr0 = constp.tile([P, 1], F32, tag="scr0")
            scr1 = constp.tile([P, 1], F32, tag="scr1")
            outrow = constp.tile([1, M], F32, tag="outrow")
            # 4 batch buffers: 3-deep prefetch of 8-step diagonal loads
            ambufs = [
                constp.tile([P, 8 * 2 * SEG], F32, tag=f"am{i}", name=f"am{i}")
                for i in range(4)
            ]
            gbufs = [
                constp.tile([P, 8 * 33 + 1], F32, tag=f"g{i}", name=f"g{i}")
                for i in range(4)
            ]

            nc.sync.dma_start(xst_sb[:], xst_d[:])
            nc.sync.dma_start(yt_sb[:], yt_d[:])
            nc.sync.dma_start(
                xxs_sb[:], xxs_d[:].rearrange("(b p) -> p b", p=128)
            )
            nc.sync.dma_start(yst_sb[:], yst_d[:])
            nc.sync.dma_start(xt_sb[:], xt_d[:])
            nc.sync.dma_start(xx_sb[:], xx_d[:].rearrange("(b p) -> p b", p=128))
            nc.sync.dma_start(
                xxs4_sb[:], xxs4_d[:].rearrange("(b p) -> p b", p=128)
            )
            nc.sync.dma_start(shift_sb[:], shiftm_d[:])
            nc.sync.dma_start(
                initcol[:], initc_d[:].rearrange("(p f) -> p f", f=1)
            )
            nc.gpsimd.memset(bigpad[:], BIG)
            nc.gpsimd.memset(dbuf0[:], BIG)
            nc.gpsimd.memset(dbuf1[:], BIG)
            nc.gpsimd.memset(btile[:, 0:1], BIG)
            for gb in gbufs:
                nc.gpsimd.memset(
                    gb[:, 0:264].rearrange("p (k f) -> p k f", f=33)[:, :, 0:1],
                    0.0,
                )

            # BIG padding for head slots [0,127) and tail slots [127+dp_rows,
            # N_SLOTS): garbage wavefront cells read/write values in
            # [BIG, ~130*BIG] — finite, and always losing min() against real
            # path values.
            def fill_pad(slot0, slot1):
                total = (slot1 - slot0) * SLOT  # multiple of 1536 (SLOT is)
                off = slot0 * SLOT
                chunk = 128 * 1536
                while total > 0:
                    n = min(chunk, total)
                    rows = n // 1536
                    nc.sync.dma_start(
                        SI[off:off + n].rearrange("(p f) -> p f", f=1536),
                        bigpad[0:rows, :],
                    )
                    off += n
                    total -= n

            assert SLOT % 1536 == 0
            fill_pad(0, 127)

            # ---- phase A band: three cost matrices for rows [128b, 128b+128)
            # Band layout [128, SLOT]: cols [0,8192) interleave (2*C2, C1)
            # per column — (even, odd) = the diagonal/vertical operand pair
            # phase B consumes with one paired add — cols [8192,12288) = C3.
            # 2*C2 = sqrt(4*d^2) comes free via scale=-8 & bias 4*|xs|^2.
            # Emitted as per-step closures (one 128-col matmul+sqrt chunk)
            # so the in-order PE never buries the wavefront's shift matmuls
            # under a burst of band matmuls.
            passes = (
                (0, xst_sb, xxs_sb, yt_sb, -2.0),
                (1, xst_sb, xxs4_sb, yst_sb, -8.0),
                (2, xt_sb, xx_sb, yst_sb, -2.0),
            )

            def band_chunks(b):
                c_band = bandp.tile([128, SLOT], F32, tag="c_band", name="cb")
                am_view = c_band[:, 0:2 * M].rearrange("p (c e) -> p c e", e=2)

                def chunk(t, lhs, bias_sb, rhs, scale, cb):
                    ps = psumA.tile([128, 128], F32, tag="psA", name="psA")
                    nc.tensor.matmul(
                        ps[:],
                        lhs[:, b * 128:(b + 1) * 128],
                        rhs[:, cb * 128:(cb + 1) * 128],
                    )
                    if t == 2:
                        out = c_band[:, 2 * M + cb * 128:2 * M + (cb + 1) * 128]
                    else:
                        out = am_view[:, cb * 128:(cb + 1) * 128, (1 - t):(2 - t)]
                    nc.scalar.activation(
                        out, ps[:], mybir.ActivationFunctionType.Sqrt,
                        bias=bias_sb[:, b:b + 1], scale=scale,
                    )

                def write(qtr):
                    si_dst = bass.AP(
                        SI[:].tensor,
                        (b * 128 + qtr * 32 + 127) * SLOT,
                        [[SLOT, 32], [1, SLOT]],
                    )
                    nc.sync.dma_start(
                        si_dst, c_band[qtr * 32:(qtr + 1) * 32, :]
                    )

                for t, lhs, bias_sb, rhs, scale in passes:
                    for cb in range(32):
                        yield (lambda t=t, lhs=lhs, bias_sb=bias_sb, rhs=rhs,
                               scale=scale, cb=cb: chunk(t, lhs, bias_sb, rhs,
                                                         scale, cb))
                for qtr in range(4):
                    yield (lambda qtr=qtr: write(qtr))

            def emit_band(b):
                for fn in band_chunks(b):
                    fn()

            # ---- phase B batch load: 8 diagonal steps per DMA trio ----
            # partition p <-> segment q=127-p; step s, partition p reads
            # (row s-127+p, seg q) = slot s+p at column offset 32q.
            def emit_load(batch):
                S = batch * 8
                amt = ambufs[batch % 4]
                gt_ = gbufs[batch % 4]
                nc.sync.dma_start(
                    amt[:].rearrange("p (k f) -> p k f", f=2 * SEG),
                    bass.AP(
                        SI[:].tensor, S * SLOT + (P - 1) * 2 * SEG,
                        [[SLOT - 2 * SEG, 128], [SLOT, 8], [1, 2 * SEG]],
                    ),
                )
                nc.sync.dma_start(
                    gt_[:, 1:265].rearrange("p (k f) -> p k f", f=33)[:, :, 0:32],
                    bass.AP(
                        SI[:].tensor, S * SLOT + 2 * M + (P - 1) * SEG,
                        [[SLOT - SEG, 128], [SLOT, 8], [1, SEG]],
                    ),
                )

            # Band 0 is on the startup critical path: its chunks run upfront, and only the quarter-0 writes (rows
            # 0-31 — all the first ~3 loads read) go ahead of the loads.
            c_band0 = bandp.tile([128, SLOT], F32, tag="c_band", name="cb0")
            am_view0 = c_band0[:, 0:2 * M].rearrange("p (c e) -> p c e", e=2)
            for t, lhs, bias_sb, rhs, scale in passes:
                for cb in range(32):
                    ps = psumA.tile([128, 128], F32, tag="psA", name="psA")
                    nc.tensor.matmul(
                        ps[:], lhs[:, 0:128], rhs[:, cb * 128:(cb + 1) * 128]
                    )
                    if t == 2:
                        out = c_band0[:, 2 * M + cb * 128:2 * M + (cb + 1) * 128]
                    else:
                        out = am_view0[:, cb * 128:(cb + 1) * 128, (1 - t):(2 - t)]
                    nc.scalar.activation(
                        out, ps[:], mybir.ActivationFunctionType.Sqrt,
                        bias=bias_sb[:, 0:1], scale=scale,
                    )
                if t == 1:
                    nc.sync.dma_start(
                        bass.AP(SI[:].tensor, 127 * SLOT, [[SLOT, 32], [1, 2 * M]]),
                        c_band0[0:32, 0:2 * M],
                    )
            nc.sync.dma_start(
                bass.AP(SI[:].tensor, 127 * SLOT + 2 * M, [[SLOT, 32], [1, M]]),
                c_band0[0:32, 2 * M:SLOT],
            )
            # Row 0's boundary column carries the virtual D[0,-1]=0 cumsum
            # seed, which row 1's diagonal term must NOT see as a real cell:
            # overwrite 2*C2[1,0] with BIG so that single path is cut exactly.
            nc.sync.dma_start(
                SI[128 * SLOT:128 * SLOT + 1].rearrange("(p f) -> p f", f=1),
                bigpad[0:1, 0:1],
            )
            emit_load(0)
            emit_load(1)
            emit_load(2)

            def band0_write(qtr):
                nc.sync.dma_start(
                    bass.AP(
                        SI[:].tensor, (qtr * 32 + 127) * SLOT,
                        [[SLOT, 32], [1, SLOT]],
                    ),
                    c_band0[qtr * 32:(qtr + 1) * 32, :],
                )

            for qtr in range(1, 4):
                band0_write(qtr)

            # ---- phase B: diagonal wavefront ----
            # Remaining bands drip into the step stream, two 128-col
            # matmul+sqrt chunks per step (~2.5 rows produced per row
            # consumed) so loads never wait on band writes; band 1 gets a
            # head start during the startup window (PE is idle then)
            pending = []
            for b in range(1, n_bands):
                pending.extend(band_chunks(b))
            pending.reverse()  # pop() from the tail emits in order

            dbufs = [dbuf0, dbuf1]
            scrs = [scr0, scr1]
            dtens = [dbuf0[:].tensor, dbuf1[:].tensor]
            ps_prev = None
            for s in range(steps):
                if s >= 32:
                    # drip starts after the wavefront is rolling: the
                    # scheduler hoists ready chunk matmuls ahead of blocked
                    # shifts, and early scans (counting sems) would wait for
                    # that whole Act-paced backlog
                    for _ in range(2):
                        if pending:
                            pending.pop()()
                if s == 1024:
                    fill_pad(127 + dp_rows, N_SLOTS)
                amt = ambufs[(s // 8) % 4]
                gt_ = gbufs[(s // 8) % 4]
                sl = s % 8
                am_s = amt[:, sl * 2 * SEG:(sl + 1) * 2 * SEG]
                g33 = gt_[:, sl * 33:sl * 33 + 33]
                dcur = dbufs[s % 2]
                dprev = dbufs[(s + 1) % 2]
                # paired operands (D[r-1,j-1], D[r-1,j]) via overlapping AP
                dpairs = bass.AP(
                    dtens[(s + 1) % 2], 0, [[SEG + 1, 128], [1, SEG], [1, 2]]
                )
                nc.vector.tensor_tensor(
                    uint_t[:].rearrange("p (c e) -> p c e", e=2),
                    dpairs, am_s.rearrange("p (c e) -> p c e", e=2),
                    op=AluOp.add,
                )
                if s > 0:
                    # stage the init (shift_{s-1}'s PSUM result, ready since
                    # last step) into SBUF here: this copy's waits are all
                    # satisfied, so it absorbs the 2-wait-class
                    # EventSemaphore cheaply and the scan keeps a single
                    # DVE-class wait — no +100ns ES on the critical tail
                    nc.vector.tensor_copy(scrs[s % 2][:, 0:1], ps_prev[:, 0:1])
                nc.vector.tensor_reduce(
                    btile[:, 1:SEG + 1],
                    uint_t[:].rearrange("p (c e) -> p c e", e=2),
                    axis=mybir.AxisListType.X, op=AluOp.min,
                )
                init = initcol[:, 0:1] if s == 0 else scrs[s % 2][:, 0:1]
                nc.vector.tensor_tensor_scan(
                    dcur[:, 0:SEG + 1], g33, btile[:, 0:SEG + 1], init,
                    op0=AluOp.add, op1=AluOp.min,
                )
                if s + 1 < steps:
                    ps_cur = psumS.tile([128, 1], F32, tag="shift")
                    nc.tensor.matmul(
                        ps_cur[:], shift_sb[:, 0:128], dcur[:, SEG:SEG + 1]
                    )
                    ps_prev = ps_cur
                if s % 8 == 0 and s // 8 + 3 < n_batches:
                    # end-of-step emission: batch-boundary scans wait (via
                    # the per-queue counting sems) on every earlier same-
                    # queue DMA, so never put a fresh load ahead of a scan
                    emit_load(s // 8 + 3)
                if s >= dp_rows - 1:
                    # extract partition p's last-row segment via a one-hot
                    # PE matmul (engine ops can't start at partition p)
                    p = dp_rows - 1 + P - 1 - s
                    q = P - 1 - p
                    pse = psumS.tile([128, SEG], F32, tag="ext")
                    nc.tensor.matmul(
                        pse[0:1, 0:SEG],
                        shift_sb[:, 128 + p:129 + p],
                        dcur[:, 1:SEG + 1],
                    )
                    nc.vector.tensor_copy(
                        outrow[0:1, q * SEG:(q + 1) * SEG], pse[0:1, 0:SEG]
                    )

            nc.sync.dma_start(
                out_d[:].rearrange("(p f) -> p f", p=1), outrow[:]
            )

    nc.compile()
    _nc_cache[key] = nc
    return nc


def _aug_t(a):
    """[n, 64] -> [65, n] transposed + ones row (f32)."""
    n = a.shape[0]
    t = np.ones((65, n), np.float32)
    t[:64] = a.T
    return t


def _aug_y(yf):
    """[m, 64] -> [65, m]: y^T with row 64 = -0.5*(|y|^2 + 1e-12)."""
    t = np.empty((65, yf.shape[0]), np.float32)
    t[:64] = yf.T
    yy = (yf.astype(np.float64) ** 2).sum(1)
    t[64] = (-0.5 * (yy + 1e-12)).astype(np.float32)
    return t


def _pe_consts():
    """[:, 0:128]: lhsT so out[m] = ends[m+1]; partition 127 (segment 0)
    instead gets BIGW * ends[0] ~ +inf (its left boundary is outside the
    matrix). [:, 128:256]: identity for one-hot row extraction."""
    sm = np.zeros((128, 256), np.float32)
    for m_ in range(127):
        sm[m_ + 1, m_] = 1.0
    sm[0, 127] = BIGW
    sm[:, 128:256] = np.eye(128, dtype=np.float32)
    return sm


def _init_col():
    col = np.full(128, BIG, np.float32)
    col[127] = 0.0  # virtual D[0,-1] = 0 seeds row 0's cumsum
    return col


def _host_prep(xh, yf, core):
    if core == 0:
        xs, ys = xh, yf
    else:
        xs = np.concatenate([np.zeros((1, DIM), np.float32), xh[:-1]])
        # ys[0] pairs with x[0] to make C3[0,0] ~ 0 (H[0,0]=0). Perturb it
        # so the fp32-cancelled squared distance stays safely positive
        # (sqrt of a slightly negative value would poison the DP with NaN);
        # the 0.1 offset costs ~1e-6 relative error on the final answer.
        y0 = xh[0:1].copy()
        y0[0, 0] += 0.1
        ys = np.concatenate([y0, yf[:-1]])
    xxs = (xs.astype(np.float64) ** 2).sum(1)
    return {
        "xt": _aug_t(xh),
        "xst": _aug_t(xs),
        "yt": _aug_y(yf),
        "yst": _aug_y(ys),
        "xx": (xh.astype(np.float64) ** 2).sum(1).astype(np.float32),
        "xxs": xxs.astype(np.float32),
        "xxs4": (4.0 * xxs).astype(np.float32),
        "shiftm": _pe_consts(),
        "initc": _init_col(),
    }


def kernel(x, y):
    x = np.ascontiguousarray(np.asarray(x, dtype=np.float32))
    y = np.ascontiguousarray(np.asarray(y, dtype=np.float32))
    assert x.shape == (N, DIM) and y.shape == (M, DIM)

    from concourse.bass_utils import run_bass_kernel_spmd

    nc = _build_nc()
    in_maps = [
        _host_prep(x[:N_ROWS], y, core=0),
        _host_prep(x[::-1][:N_ROWS].copy(), y[::-1].copy(), core=1),
    ]
    res = run_bass_kernel_spmd(nc, in_maps, core_ids=[0, 1])
    F_last = res.results[0]["out"].astype(np.float64)
    H_last = res.results[1]["out"].astype(np.float64)

    # host merge across the row-2047/2048 seam
    xm = x[N_ROWS].astype(np.float64)
    sq = (xm * xm).sum() + (y.astype(np.float64) ** 2).sum(1) - 2.0 * (
        y.astype(np.float64) @ xm
    )
    c_mid = np.sqrt(np.maximum(sq, 1e-12))
    B_row = H_last[::-1]
    cand_v = F_last + c_mid + B_row
    cand_d = F_last[:-1] + 2.0 * c_mid[1:] + B_row[1:]
    ans = min(cand_v.min(), cand_d.min())
    return np.float32(ans)


# revision 40
# speedup vs baseline: 1.0127x; 1.0016x over previous
"""DTW loss (symmetric2, unnormalized) on trn2 — bidirectional 2-core kernel
with a partition-skewed diagonal-wavefront DP.

kernel(x, y) -> np.float32 scalar DTW distance, matching the jax reference:
  D[0,0]=c[0,0]; D[0,j]=D[0,j-1]+c; D[i,0]=D[i-1,0]+c
  D[i,j]=min(D[i-1,j-1]+2c, D[i-1,j]+c, D[i,j-1]+c);  c = euclidean cdist.

Core 0 runs the forward DP over rows [0,2048); core 1 runs the backward
table H (reverse DP, entering-convention) on reversed inputs. Both run the
SAME program (see _host_prep for the shifted-cost-matrix trick). The host
merges the two meeting rows.

Phase A computes the cost matrices by PE matmul (K=65 augmented with norm
rows) + ScalarE sqrt, staged to DRAM as [slot][12288] with slot = row+127
(padding slots hold BIG): cols [0,8192) interleave (2*C2, C1) per column
(2*C2 = sqrt(4 d^2) via scale=-8/bias 4|xs|^2), cols [8192,12288) = C3.

Phase B is a diagonal wavefront: at step s, partition p handles segment
q = 127-p (columns [32q, 32q+32)) of DP row r = s-127+p. The 32-wide
min-plus scan per partition is exact given a per-partition init (the left
boundary D[r, 32q-1]), which equals the segment-end of partition p+1 at
step s-1. That cross-partition move runs on the otherwise-idle PE as a
subdiagonal-matrix matmul into PSUM; the next scan reads its init straight
from PSUM. The scan covers 33 elements: element 0 re-emits the init into
the D buffer's boundary column (data0=0, data1=BIG there), so the next
row's paired read (D[r-1,j-1], D[r-1,j]) is one overlapping-stride AP.
Per step the DVE runs only 3 chained ops — paired add, pairwise
tensor_reduce(min), scan — at 733 ns/step vs the 9-op/1857 ns row loop of
the row-sequential version. That is this structure's latency floor: the
chain (3 sem hops + 3 ops) costs 633 ns and the scan's second wait class
(PSUM init + same-engine RAW, ISA allows one inline wait) forces a
split EventSemaphore worth +100 ns; the PE round trip itself is fully
hidden. Phase A drips into the step stream as 128-col matmul+sqrt chunks
(two per step) and loads prefetch 3 batches ahead, so the in-order
PE/SP/DMA engines never head-of-line-block the wavefront.
"""

import sys

sys.path.insert(0, "/opt/trn_rl_repo")

import numpy as np

N = 4096          # rows of x
M = 4096          # rows of y
DIM = 64
N_ROWS = N // 2   # DP rows per core
P = 128           # partitions in the wavefront
SEG = 32          # columns per segment (M // P)
STEPS = N_ROWS + P - 1
SLOT = 3 * M      # floats per DP-row slot in SI
N_SLOTS = P + N_ROWS + 129  # 127 head pads, rows, tail pads (incl. load overrun)
BIG = 1e30
BIGW = 1e36       # shift-matrix weight that maps any end value to ~inf

_nc_cache = {}


def _build_nc(n_rows=N_ROWS, dp_rows=None):
    """dp_rows: run phase B over only this many rows (timing experiments)."""
    if dp_rows is None:
        dp_rows = n_rows
    key = (n_rows, dp_rows)
    if key in _nc_cache:
        return _nc_cache[key]
    import concourse.bacc as bacc
    import concourse.bass as bass
    import concourse.mybir as mybir
    from concourse import tile

    F32 = mybir.dt.float32
    AluOp = mybir.AluOpType
    n_bands = n_rows // 128
    steps = dp_rows + P - 1
    n_batches = (steps + 7) // 8

    nc = bacc.Bacc(None, target_bir_lowering=False)

    xt_d = nc.dram_tensor("xt", [65, n_rows], F32, kind="ExternalInput")
    xst_d = nc.dram_tensor("xst", [65, n_rows], F32, kind="ExternalInput")
    yt_d = nc.dram_tensor("yt", [65, M], F32, kind="ExternalInput")
    yst_d = nc.dram_tensor("yst", [65, M], F32, kind="ExternalInput")
    xx_d = nc.dram_tensor("xx", [n_rows], F32, kind="ExternalInput")
    xxs_d = nc.dram_tensor("xxs", [n_rows], F32, kind="ExternalInput")
    xxs4_d = nc.dram_tensor("xxs4", [n_rows], F32, kind="ExternalInput")
    # [:, 0:128] subdiagonal shift (+BIGW hook), [:, 128:256] identity
    shiftm_d = nc.dram_tensor("shiftm", [128, 256], F32, kind="ExternalInput")
    initc_d = nc.dram_tensor("initc", [128], F32, kind="ExternalInput")
    out_d = nc.dram_tensor("out", [M], F32, kind="ExternalOutput")
    SI = nc.dram_tensor("SI", [N_SLOTS * SLOT], F32)

    with tile.TileContext(nc) as tc:
        with (
            tc.tile_pool(name="const", bufs=1) as constp,
            tc.tile_pool(name="band", bufs=2) as bandp,
            tc.tile_pool(name="psumA", bufs=5, space="PSUM") as psumA,
            tc.tile_pool(name="psumS", bufs=2, space="PSUM") as psumS,
            tc.tile_pool(name="psumE", bufs=1, space="PSUM") as psumE,
        ):
            xt_sb = constp.tile([65, n_rows], F32, tag="xt")
            xst_sb = constp.tile([65, n_rows], F32, tag="xst")
            yt_sb = constp.tile([65, M], F32, tag="yt")
            yst_sb = constp.tile([65, M], F32, tag="yst")
            xx_sb = constp.tile([128, n_bands], F32, tag="xx")
            xxs_sb = constp.tile([128, n_bands], F32, tag="xxs")
            xxs4_sb = constp.tile([128, n_bands], F32, tag="xxs4")
            shift_sb = constp.tile([128, 256], F32, tag="shiftm")
            bigpad = constp.tile([128, 1536], F32, tag="bigpad")
            initcol = constp.tile([P, 1], F32, tag="initcol")
            uint_t = constp.tile([P, 2 * SEG], F32, tag="uint_t")
            btile = constp.tile([P, SEG + 1], F32, tag="btile")
            dbuf0 = constp.tile([P, SEG + 1], F32, tag="dbuf0")
            dbuf1 = constp.tile([P, SEG + 1], F32, tag="dbuf1")
            scr0 = constp.tile([P, 1], F32, tag="scr0")
            scr1 = constp.tile([P, 1], F32, tag="scr1")
            outrow = constp.tile([1, M], F32, tag="outrow")
            # 4 batch buffers: 3-deep prefetch of 8-step diagonal loads
            ambufs = [
                constp.tile([P, 8 * 2 * SEG], F32, tag=f"am{i}", name=f"am{i}")
                for i in range(4)
            ]
            gbufs = [
                constp.tile([P, 8 * 33 + 1], F32, tag=f"g{i}", name=f"g{i}")
                for i in range(4)
            ]

            nc.sync.dma_start(xst_sb[:], xst_d[:])
            nc.sync.dma_start(yt_sb[:], yt_d[:])
            nc.sync.dma_start(
                xxs_sb[:], xxs_d[:].rearrange("(b p) -> p b", p=128)
            )
            nc.sync.dma_start(yst_sb[:], yst_d[:])
            nc.sync.dma_start(xt_sb[:], xt_d[:])
            nc.sync.dma_start(xx_sb[:], xx_d[:].rearrange("(b p) -> p b", p=128))
            nc.sync.dma_start(
                xxs4_sb[:], xxs4_d[:].rearrange("(b p) -> p b", p=128)
            )
            nc.sync.dma_start(shift_sb[:], shiftm_d[:])
            nc.sync.dma_start(
                initcol[:], initc_d[:].rearrange("(p f) -> p f", f=1)
            )
            nc.gpsimd.memset(bigpad[:], BIG)
            nc.gpsimd.memset(dbuf0[:], BIG)
            nc.gpsimd.memset(dbuf1[:], BIG)
            nc.gpsimd.memset(btile[:, 0:1], BIG)
            for gb in gbufs:
                nc.gpsimd.memset(
                    gb[:, 0:264].rearrange("p (k f) -> p k f", f=33)[:, :, 0:1],
                    0.0,
                )

            # BIG padding for head slots [0,127) and tail slots [127+dp_rows,
            # N_SLOTS): garbage wavefront cells read/write values in
            # [BIG, ~130*BIG] — finite, and always losing min() against real
            # path values.
            def fill_pad(slot0, slot1):
                total = (slot1 - slot0) * SLOT  # multiple of 1536 (SLOT is)
                off = slot0 * SLOT
                chunk = 128 * 1536
                while total > 0:
                    n = min(chunk, total)
                    rows = n // 1536
                    nc.sync.dma_start(
                        SI[off:off + n].rearrange("(p f) -> p f", f=1536),
                        bigpad[0:rows, :],
                    )
                    off += n
                    total -= n

            assert SLOT % 1536 == 0
            fill_pad(0, 127)

            # ---- phase A band: three cost matrices for rows [128b, 128b+128)
            # Band layout [128, SLOT]: cols [0,8192) interleave (2*C2, C1)
            # per column — (even, odd) = the diagonal/vertical operand pair
            # phase B consumes with one paired add — cols [8192,12288) = C3.
            # 2*C2 = sqrt(4*d^2) comes free via scale=-8 & bias 4*|xs|^2.
            # Emitted as per-step closures (one 128-col matmul+sqrt chunk)
            # so the in-order PE never buries the wavefront's shift matmuls
            # under a burst of band matmuls.
            passes = (
                (0, xst_sb, xxs_sb, yt_sb, -2.0),
                (1, xst_sb, xxs4_sb, yst_sb, -8.0),
                (2, xt_sb, xx_sb, yst_sb, -2.0),
            )

            def band_chunks(b):
                c_band = bandp.tile([128, SLOT], F32, tag="c_band", name="cb")
                am_view = c_band[:, 0:2 * M].rearrange("p (c e) -> p c e", e=2)

                def chunk(t, lhs, bias_sb, rhs, scale, cb):
                    ps = psumA.tile([128, 128], F32, tag="psA", name="psA")
                    nc.tensor.matmul(
                        ps[:],
                        lhs[:, b * 128:(b + 1) * 128],
                        rhs[:, cb * 128:(cb + 1) * 128],
                    )
                    if t == 2:
                        out = c_band[:, 2 * M + cb * 128:2 * M + (cb + 1) * 128]
                    else:
                        out = am_view[:, cb * 128:(cb + 1) * 128, (1 - t):(2 - t)]
                    nc.scalar.activation(
                        out, ps[:], mybir.ActivationFunctionType.Sqrt,
                        bias=bias_sb[:, b:b + 1], scale=scale,
                    )

                def write(qtr):
                    si_dst = bass.AP(
                        SI[:].tensor,
                        (b * 128 + qtr * 32 + 127) * SLOT,
                        [[SLOT, 32], [1, SLOT]],
                    )
                    nc.sync.dma_start(
                        si_dst, c_band[qtr * 32:(qtr + 1) * 32, :]
                    )

                for t, lhs, bias_sb, rhs, scale in passes:
                    for cb in range(32):
                        yield (lambda t=t, lhs=lhs, bias_sb=bias_sb, rhs=rhs,
                               scale=scale, cb=cb: chunk(t, lhs, bias_sb, rhs,
                                                         scale, cb))
                for qtr in range(4):
                    yield (lambda qtr=qtr: write(qtr))

            def emit_band(b):
                for fn in band_chunks(b):
                    fn()

            # ---- phase B batch load: 8 diagonal steps per DMA trio ----
            # partition p <-> segment q=127-p; step s, partition p reads
            # (row s-127+p, seg q) = slot s+p at column offset 32q.
            def emit_load(batch):
                S = batch * 8
                amt = ambufs[batch % 4]
                gt_ = gbufs[batch % 4]
                nc.sync.dma_start(
                    amt[:].rearrange("p (k f) -> p k f", f=2 * SEG),
                    bass.AP(
                        SI[:].tensor, S * SLOT + (P - 1) * 2 * SEG,
                        [[SLOT - 2 * SEG, 128], [SLOT, 8], [1, 2 * SEG]],
                    ),
                )
                nc.sync.dma_start(
                    gt_[:, 1:265].rearrange("p (k f) -> p k f", f=33)[:, :, 0:32],
                    bass.AP(
                        SI[:].tensor, S * SLOT + 2 * M + (P - 1) * SEG,
                        [[SLOT - SEG, 128], [SLOT, 8], [1, SEG]],
                    ),
                )

            # Band 0 is on the startup critical path: its chunks run upfront, and only the quarter-0 writes (rows
            # 0-31 — all the first ~3 loads read) go ahead of the loads.
            c_band0 = bandp.tile([128, SLOT], F32, tag="c_band", name="cb0")
            am_view0 = c_band0[:, 0:2 * M].rearrange("p (c e) -> p c e", e=2)
            for t, lhs, bias_sb, rhs, scale in passes:
                for cb in range(32):
                    ps = psumA.tile([128, 128], F32, tag="psA", name="psA")
                    nc.tensor.matmul(
                        ps[:], lhs[:, 0:128], rhs[:, cb * 128:(cb + 1) * 128]
                    )
                    if t == 2:
                        out = c_band0[:, 2 * M + cb * 128:2 * M + (cb + 1) * 128]
                    else:
                        out = am_view0[:, cb * 128:(cb + 1) * 128, (1 - t):(2 - t)]
                    nc.scalar.activation(
                        out, ps[:], mybir.ActivationFunctionType.Sqrt,
                        bias=bias_sb[:, 0:1], scale=scale,
                    )
                if t == 1:
                    nc.sync.dma_start(
                        bass.AP(SI[:].tensor, 127 * SLOT, [[SLOT, 32], [1, 2 * M]]),
                        c_band0[0:32, 0:2 * M],
                    )
            nc.sync.dma_start(
                bass.AP(SI[:].tensor, 127 * SLOT + 2 * M, [[SLOT, 32], [1, M]]),
                c_band0[0:32, 2 * M:SLOT],
            )
            # Row 0's boundary column carries the virtual D[0,-1]=0 cumsum
            # seed, which row 1's diagonal term must NOT see as a real cell:
            # overwrite 2*C2[1,0] with BIG so that single path is cut exactly.
            nc.sync.dma_start(
                SI[128 * SLOT:128 * SLOT + 1].rearrange("(p f) -> p f", f=1),
                bigpad[0:1, 0:1],
            )
            emit_load(0)
            emit_load(1)
            emit_load(2)

            def band0_write(qtr):
                nc.sync.dma_start(
                    bass.AP(
                        SI[:].tensor, (qtr * 32 + 127) * SLOT,
                        [[SLOT, 32], [1, SLOT]],
                    ),
                    c_band0[qtr * 32:(qtr + 1) * 32, :],
                )

            for qtr in range(1, 4):
                band0_write(qtr)

            # ---- phase B: diagonal wavefront ----
            # Remaining bands drip into the step stream, two 128-col
            # matmul+sqrt chunks per step (~2.5 rows produced per row
            # consumed) so loads never wait on band writes; band 1 gets a
            # head start during the startup window (PE is idle then)
            pending = []
            for b in range(1, n_bands):
                pending.extend(band_chunks(b))
            pending.reverse()  # pop() from the tail emits in order

            dbufs = [dbuf0, dbuf1]
            scrs = [scr0, scr1]
            dtens = [dbuf0[:].tensor, dbuf1[:].tensor]
            ps_prev = None
            for s in range(steps):
                if s >= 32:
                    # drip starts after the wavefront is rolling: the
                    # scheduler hoists ready chunk matmuls ahead of blocked
                    # shifts, and early scans (counting sems) would wait for
                    # that whole Act-paced backlog
                    for _ in range(2):
                        if pending:
                            pending.pop()()
                if s == 1024:
                    fill_pad(127 + dp_rows, N_SLOTS)
                amt = ambufs[(s // 8) % 4]
                gt_ = gbufs[(s // 8) % 4]
                sl = s % 8
                am_s = amt[:, sl * 2 * SEG:(sl + 1) * 2 * SEG]
                g33 = gt_[:, sl * 33:sl * 33 + 33]
                dcur = dbufs[s % 2]
                dprev = dbufs[(s + 1) % 2]
                # paired operands (D[r-1,j-1], D[r-1,j]) via overlapping AP
                dpairs = bass.AP(
                    dtens[(s + 1) % 2], 0, [[SEG + 1, 128], [1, SEG], [1, 2]]
                )
                nc.vector.tensor_tensor(
                    uint_t[:].rearrange("p (c e) -> p c e", e=2),
                    dpairs, am_s.rearrange("p (c e) -> p c e", e=2),
                    op=AluOp.add,
                )
                if s > 0:
                    # stage the init (shift_{s-1}'s PSUM result, ready since
                    # last step) into SBUF here: this copy's waits are all
                    # satisfied, so it absorbs the 2-wait-class
                    # EventSemaphore cheaply and the scan keeps a single
                    # DVE-class wait — no +100ns ES on the critical tail
                    nc.vector.tensor_copy(scrs[s % 2][:, 0:1], ps_prev[:, 0:1])
                nc.vector.tensor_reduce(
                    btile[:, 1:SEG + 1],
                    uint_t[:].rearrange("p (c e) -> p c e", e=2),
                    axis=mybir.AxisListType.X, op=AluOp.min,
                )
                init = initcol[:, 0:1] if s == 0 else scrs[s % 2][:, 0:1]
                nc.vector.tensor_tensor_scan(
                    dcur[:, 0:SEG + 1], g33, btile[:, 0:SEG + 1], init,
                    op0=AluOp.add, op1=AluOp.min,
                )
                if s + 1 < steps:
                    ps_cur = psumS.tile([128, 1], F32, tag="shift")
                    nc.tensor.matmul(
                        ps_cur[:], shift_sb[:, 0:128], dcur[:, SEG:SEG + 1]
                    )
                    ps_prev = ps_cur
                if s % 8 == 0 and s // 8 + 3 < n_batches:
                    # end-of-step emission: batch-boundary scans wait (via
                    # the per-queue counting sems) on every earlier same-
                    # queue DMA, so never put a fresh load ahead of a scan
                    emit_load(s // 8 + 3)
                if s >= dp_rows - 1:
                    # extract partition p's last-row segment via a one-hot
                    # PE matmul (engine ops can't start at partition p)
                    p = dp_rows - 1 + P - 1 - s
                    q = P - 1 - p
                    pse = psumE.tile([128, SEG], F32, tag="ext")
                    nc.tensor.matmul(
                        pse[0:1, 0:SEG],
                        shift_sb[:, 128 + p:129 + p],
                        dcur[:, 1:SEG + 1],
                    )
                    nc.vector.tensor_copy(
                        outrow[0:1, q * SEG:(q + 1) * SEG], pse[0:1, 0:SEG]
                    )

            nc.sync.dma_start(
                out_d[:].rearrange("(p f) -> p f", p=1), outrow[:]
            )

    nc.compile()
    _nc_cache[key] = nc
    return nc


def _aug_t(a):
    """[n, 64] -> [65, n] transposed + ones row (f32)."""
    n = a.shape[0]
    t = np.ones((65, n), np.float32)
    t[:64] = a.T
    return t


def _aug_y(yf):
    """[m, 64] -> [65, m]: y^T with row 64 = -0.5*(|y|^2 + 1e-12)."""
    t = np.empty((65, yf.shape[0]), np.float32)
    t[:64] = yf.T
    yy = (yf.astype(np.float64) ** 2).sum(1)
    t[64] = (-0.5 * (yy + 1e-12)).astype(np.float32)
    return t


def _pe_consts():
    """[:, 0:128]: lhsT so out[m] = ends[m+1]; partition 127 (segment 0)
    instead gets BIGW * ends[0] ~ +inf (its left boundary is outside the
    matrix). [:, 128:256]: identity for one-hot row extraction."""
    sm = np.zeros((128, 256), np.float32)
    for m_ in range(127):
        sm[m_ + 1, m_] = 1.0
    sm[0, 127] = BIGW
    sm[:, 128:256] = np.eye(128, dtype=np.float32)
    return sm


def _init_col():
    col = np.full(128, BIG, np.float32)
    col[127] = 0.0  # virtual D[0,-1] = 0 seeds row 0's cumsum
    return col


def _host_prep(xh, yf, core):
    if core == 0:
        xs, ys = xh, yf
    else:
        xs = np.concatenate([np.zeros((1, DIM), np.float32), xh[:-1]])
        # ys[0] pairs with x[0] to make C3[0,0] ~ 0 (H[0,0]=0). Perturb it
        # so the fp32-cancelled squared distance stays safely positive
        # (sqrt of a slightly negative value would poison the DP with NaN);
        # the 0.1 offset costs ~1e-6 relative error on the final answer.
        y0 = xh[0:1].copy()
        y0[0, 0] += 0.1
        ys = np.concatenate([y0, yf[:-1]])
    xxs = (xs.astype(np.float64) ** 2).sum(1)
    return {
        "xt": _aug_t(xh),
        "xst": _aug_t(xs),
        "yt": _aug_y(yf),
        "yst": _aug_y(ys),
        "xx": (xh.astype(np.float64) ** 2).sum(1).astype(np.float32),
        "xxs": xxs.astype(np.float32),
        "xxs4": (4.0 * xxs).astype(np.float32),
        "shiftm": _pe_consts(),
        "initc": _init_col(),
    }


def kernel(x, y):
    x = np.ascontiguousarray(np.asarray(x, dtype=np.float32))
    y = np.ascontiguousarray(np.asarray(y, dtype=np.float32))
    assert x.shape == (N, DIM) and y.shape == (M, DIM)

    from concourse.bass_utils import run_bass_kernel_spmd

    nc = _build_nc()
    in_maps = [
        _host_prep(x[:N_ROWS], y, core=0),
        _host_prep(x[::-1][:N_ROWS].copy(), y[::-1].copy(), core=1),
    ]
    res = run_bass_kernel_spmd(nc, in_maps, core_ids=[0, 1])
    F_last = res.results[0]["out"].astype(np.float64)
    H_last = res.results[1]["out"].astype(np.float64)

    # host merge across the row-2047/2048 seam
    xm = x[N_ROWS].astype(np.float64)
    sq = (xm * xm).sum() + (y.astype(np.float64) ** 2).sum(1) - 2.0 * (
        y.astype(np.float64) @ xm
    )
    c_mid = np.sqrt(np.maximum(sq, 1e-12))
    B_row = H_last[::-1]
    cand_v = F_last + c_mid + B_row
    cand_d = F_last[:-1] + 2.0 * c_mid[1:] + B_row[1:]
    ans = min(cand_v.min(), cand_d.min())
    return np.float32(ans)
